# revision 1
# baseline (speedup 1.0000x reference)
# Multi-head attention (B=2, S=2048, D=1024, H=16) on 8 TRN2 NeuronCores.
#
# Sharding (hardcoded): core c in [0..8) handles batch b = c//4 and head
# group g = c%4 (4 heads = 256 output features of wq/wk/wv, 256 input rows
# of wo). Each core computes a partial output projection [S, D]; the host
# sums the 4 partials per batch and adds wo_bias (row-parallel unshard).
#
# Device-side layout choices:
#   - activations enter transposed ([D, S]) so every matmul contracts over
#     the partition axis with no on-device transposes;
#   - scores are computed transposed (S^T[k, q]) so softmax(P) feeds the
#     P@V matmul directly (contraction over k on partitions);
#   - the softmax denominator comes free as an extra ones-column appended
#     to each head's V block (output row 64 of the PV accumulation);
#   - matmuls run in float32r (full-rate fp32 path for moving dim >= 256);
#     P/V/out-proj run in bf16.
import functools
import sys

import numpy as np

try:
    import concourse  # noqa: F401
except ImportError:  # harness env without the default path
    sys.path.insert(0, "/opt/trn_rl_repo")
    sys.path.insert(0, "/opt/pypackages")

import ml_dtypes

BF16 = ml_dtypes.bfloat16

B, S, D, H = 2, 2048, 1024, 16
HD = D // H          # 64
NCORES = 8
GH = 4               # head groups (tensor-parallel)
HPG = H // GH        # heads per group = 4
DG = D // GH         # features per group = 256
P = 128              # partitions
TDIN = D // P        # 8 din tiles
SC = 4               # s-chunks of 512 for projections
CW = S // SC         # 512
QC = 2               # q-chunks of 1024 for attention
QW = S // QC         # 1024
KT = S // P          # 16 k tiles
NT2 = DG // P        # 2 dout tiles per group


def build_graph():
    """Build the SPMD Bass graph (identical on all 8 cores)."""
    from contextlib import ExitStack

    from concourse import bacc, mybir, tile

    f32 = mybir.dt.float32
    f32r = mybir.dt.float32r
    bf16 = mybir.dt.bfloat16
    EXP = mybir.ActivationFunctionType.Exp

    nc = bacc.Bacc(
        "TRN2", target_bir_lowering=False, debug=False, num_devices=NCORES
    )

    xq = nc.dram_tensor("xq_t", (P, TDIN, S), bf16, kind="ExternalInput")
    xk = nc.dram_tensor("xk_t", (P, TDIN, S), bf16, kind="ExternalInput")
    xv = nc.dram_tensor("xv_t", (P, TDIN, S), bf16, kind="ExternalInput")
    mk = nc.dram_tensor("mask_t", (S, S), bf16, kind="ExternalInput")
    wq = nc.dram_tensor("wq", (P, TDIN, DG), bf16, kind="ExternalInput")
    wk = nc.dram_tensor("wk", (P, TDIN, DG), bf16, kind="ExternalInput")
    wv = nc.dram_tensor("wv", (P, TDIN, DG), bf16, kind="ExternalInput")
    # wo pre-arranged host-side to [64, HPG, D] (j, h, n) so each head's
    # 64 rows sit on partitions 0..63.
    wo = nc.dram_tensor("wo", (HD, HPG, D), bf16, kind="ExternalInput")
    qb = nc.dram_tensor("qb", (1, DG), bf16, kind="ExternalInput")
    kb = nc.dram_tensor("kb", (1, DG), bf16, kind="ExternalInput")
    vb = nc.dram_tensor("vb", (1, DG), bf16, kind="ExternalInput")
    out = nc.dram_tensor("out", (S, D), bf16, kind="ExternalOutput")

    with tile.TileContext(nc) as tc, ExitStack() as ctx:
        wpool = ctx.enter_context(tc.tile_pool(name="wpool", bufs=1))
        cpool = ctx.enter_context(tc.tile_pool(name="cpool", bufs=1))
        qkpool = ctx.enter_context(tc.tile_pool(name="qk", bufs=1))
        vpool = ctx.enter_context(tc.tile_pool(name="vsb", bufs=1))
        mpool = ctx.enter_context(tc.tile_pool(name="msk", bufs=1))
        ppool = ctx.enter_context(tc.tile_pool(name="ptile", bufs=3))
        spool = ctx.enter_context(tc.tile_pool(name="small", bufs=2))
        dpool = ctx.enter_context(tc.tile_pool(name="dscr", bufs=2, space="DRAM"))
        bigps = ctx.enter_context(tc.tile_pool(name="bigps", bufs=3, space="PSUM"))
        ops_pool = ctx.enter_context(tc.tile_pool(name="ops", bufs=1, space="PSUM"))

        # ---- persistent SBUF tensors -------------------------------------
        wq_sb = wpool.tile([P, TDIN, DG], bf16)
        wk_sb = wpool.tile([P, TDIN, DG], bf16)
        wv_sb = wpool.tile([P, TDIN, DG], bf16)
        for wsb_, wdr_ in ((wq_sb, wq), (wk_sb, wk), (wv_sb, wv)):
            for th_ in range(2):
                nc.sync.dma_start(
                    wsb_[:, th_ * 4 : (th_ + 1) * 4, :],
                    wdr_.ap()[:, th_ * 4 : (th_ + 1) * 4, :],
                )
        wo_sb = wpool.tile([HD, HPG, D], bf16)
        nc.sync.dma_start(wo_sb[:], wo.ap())
        qb_sb = cpool.tile([1, DG], bf16)
        kb_sb = cpool.tile([1, DG], bf16)
        vb_sb = cpool.tile([1, DG], bf16)
        nc.sync.dma_start(qb_sb[:], qb.ap())
        nc.sync.dma_start(kb_sb[:], kb.ap())
        nc.sync.dma_start(vb_sb[:], vb.ap())
        # ones: row 0 used as [1, CW] rhs / [1, P] lhsT at partition 0;
        # row 64 used as [1, HD] lhsT at partition 64 (denominator bcast).
        ones2 = cpool.tile([1, CW], bf16)
        nc.vector.memset(ones2[:], 1.0)

        qT_sb = qkpool.tile([P, NT2, S], bf16)   # q projection, transposed
        kT_sb = qkpool.tile([P, NT2, S], bf16)
        # v blocks: per k-tile, per head: [v(64) | ones] -> 65 cols
        v_sb = vpool.tile([P, KT, HPG * (HD + 1)], bf16)
        nc.vector.memset(
            v_sb[:].rearrange("p s (h x) -> p s h x", h=HPG)[:, :, :, HD : HD + 1],
            1.0,
        )
        # ---- projections -------------------------------------------------
        # q, k: out qT[dout, s] = wq^T(stationary) x q^T(moving) + bias
        xpool_cm = tc.tile_pool(name="xin", bufs=2)
        xpool = xpool_cm.__enter__()
        NCH = S // 1024
        for xdram, wsb, bias_sb, dest in (
            (xq, wq_sb, qb_sb, qT_sb),
            (xk, wk_sb, kb_sb, kT_sb),
        ):
            for sc in range(NCH):
                xch = xpool.tile([P, TDIN, 1024], bf16, tag="xch")
                for th_ in range(4):
                    nc.sync.dma_start(
                        xch[:, th_ * 2 : (th_ + 1) * 2, :],
                        xdram.ap()[
                            :, th_ * 2 : (th_ + 1) * 2, sc * 1024 : (sc + 1) * 1024
                        ],
                    )
                for half in range(2):
                    s0 = sc * 1024 + half * 512
                    for dt in range(NT2):
                        ps = bigps.tile(
                            [P, CW], f32, tag="ps", name=f"pj_{sc}_{half}_{dt}"
                        )
                        for ktl in range(TDIN):
                            nc.tensor.matmul(
                                ps[:],
                                lhsT=wsb[:, ktl, dt * P : (dt + 1) * P],
                                rhs=xch[:, ktl, half * 512 : (half + 1) * 512],
                                start=(ktl == 0),
                                stop=False,
                            )
                        nc.tensor.matmul(
                            ps[:],
                            lhsT=bias_sb[0:1, dt * P : (dt + 1) * P],
                            rhs=ones2[0:1, :],
                            start=False,
                            stop=True,
                        )
                        nc.vector.tensor_copy(
                            dest[:, dt, s0 : s0 + 512], ps[:]
                        )
        # v: natural layout [s, dout] + bias, drained per-head with ones col
        for sc in range(NCH):
            xch = xpool.tile([P, TDIN, 1024], bf16, tag="xch")
            for th_ in range(4):
                nc.sync.dma_start(
                    xch[:, th_ * 2 : (th_ + 1) * 2, :],
                    xv.ap()[
                        :, th_ * 2 : (th_ + 1) * 2, sc * 1024 : (sc + 1) * 1024
                    ],
                )
            for m in range(1024 // P):
                st = sc * (1024 // P) + m
                ps = bigps.tile([P, DG], f32, tag="ps", name=f"pv_{sc}_{m}")
                for ktl in range(TDIN):
                    nc.tensor.matmul(
                        ps[:],
                        lhsT=xch[:, ktl, m * P : (m + 1) * P],
                        rhs=wv_sb[:, ktl, :],
                        start=(ktl == 0),
                        stop=False,
                    )
                nc.tensor.matmul(
                    ps[:],
                    lhsT=ones2[0:1, 0:P],
                    rhs=vb_sb[:],
                    start=False,
                    stop=True,
                )
                nc.vector.tensor_copy(
                    v_sb[:, st, :].rearrange("p (h x) -> p h x", h=HPG)[
                        :, :, 0:HD
                    ],
                    ps[:].rearrange("p (h x) -> p h x", h=HPG),
                )
        xpool_cm.__exit__(None, None, None)

        # mask load issued after projection DMAs so it doesn't hog queues
        mask_sb = mpool.tile([P, KT, S], bf16)
        mk_r = mk.ap().rearrange("(t p) q -> p t q", p=P)
        for kt in range(KT):
            nc.sync.dma_start(mask_sb[:, kt, :], mk_r[:, kt, :])

        # ---- attention ---------------------------------------------------
        # One head at a time; score psum triple-buffered so the PE can run
        # up to 3 k-tiles ahead of the exp/mask/PV chain.
        opool_sb = ctx.enter_context(tc.tile_pool(name="otn", bufs=1))
        otn_sb = opool_sb.tile([HD, HPG, S], bf16)

        def emit_outproj(st):
            osb2 = ppool.tile([P, D], bf16, tag="outsb", name=f"outsb_{st}")
            for nch in range(2):
                op_ps = bigps.tile(
                    [P, 512], f32, tag="ps", name=f"ops2_{st}_{nch}"
                )
                for h_ in range(HPG):
                    nc.tensor.matmul(
                        op_ps[:],
                        lhsT=otn_sb[:, h_, st * P : (st + 1) * P],
                        rhs=wo_sb[:, h_, nch * 512 : (nch + 1) * 512],
                        start=(h_ == 0),
                        stop=(h_ == HPG - 1),
                    )
                nc.vector.tensor_copy(
                    osb2[:, nch * 512 : (nch + 1) * 512], op_ps[:]
                )
            nc.sync.dma_start(out.ap()[st * P : (st + 1) * P, :], osb2[:])

        pending_st = []
        for qc in range(QC):
            for h in range(HPG):
                t, po = h // 2, (h % 2) * HD
                o_ps = ops_pool.tile(
                    [HD + 1, QW], f32, tag="ops", name=f"ops_{qc}_{h}"
                )
                for kt in range(KT):
                    s_ps = bigps.tile(
                        [P, QW], f32, tag="ps", name=f"sps_{qc}_{h}_{kt}"
                    )
                    for hf in range(2):
                        nc.tensor.matmul(
                            s_ps[:, hf * 512 : (hf + 1) * 512],
                            lhsT=kT_sb[po : po + HD, t, kt * P : (kt + 1) * P],
                            rhs=qT_sb[
                                po : po + HD,
                                t,
                                qc * QW + hf * 512 : qc * QW + (hf + 1) * 512,
                            ],
                            start=True,
                            stop=True,
                        )
                    pt = ppool.tile(
                        [P, QW], bf16, tag="p", name=f"pt_{qc}_{h}_{kt}"
                    )
                    nc.scalar.activation(pt[:], s_ps[:], EXP, scale=0.125)
                    nc.vector.tensor_mul(
                        pt[:], pt[:], mask_sb[:, kt, qc * QW : (qc + 1) * QW]
                    )
                    for hf in range(2):
                        nc.tensor.matmul(
                            o_ps[:, hf * 512 : (hf + 1) * 512],
                            lhsT=v_sb[:, kt, h * 65 : (h + 1) * 65],
                            rhs=pt[:, hf * 512 : (hf + 1) * 512],
                            start=(kt == 0),
                            stop=(kt == KT - 1),
                        )
                # softmax normalization (no PE): approx-recip of the
                # denominator row, DRAM-bounce broadcast, one TT multiply.
                rec65 = spool.tile([HD + 1, QW], f32, tag="rec")
                nc.vector.reciprocal_approx_fast(out=rec65[:], in_=o_ps[:])
                osb = spool.tile([HD, QW], f32, tag="osb")
                nc.vector.tensor_copy(osb[:], o_ps[0:HD, :])
                scr = dpool.tile([1, QW], f32, tag="scr", name=f"scr_{qc}_{h}")
                nc.sync.dma_start(scr[:], rec65[HD : HD + 1, :])
                rb = spool.tile([HD, QW], f32, tag="rb")
                nc.sync.dma_start(rb[:], scr[:].to_broadcast((HD, QW)))
                nc.vector.tensor_mul(
                    otn_sb[:, h, qc * QW : (qc + 1) * QW], osb[:], rb[:]
                )

            pending_st.extend(range(qc * (QW // P), (qc + 1) * (QW // P)))

        for st in pending_st:
            emit_outproj(st)

    nc.compile()
    return nc


@functools.lru_cache(maxsize=1)
def _graph():
    return build_graph()


def make_in_maps(
    query, key, value, mask,
    wq_kernel, wq_bias, wk_kernel, wk_bias,
    wv_kernel, wv_bias, wo_kernel, wo_bias,
):
    q = np.asarray(query, np.float32)
    k = np.asarray(key, np.float32)
    v = np.asarray(value, np.float32)
    mask = np.asarray(mask)
    wqk = np.asarray(wq_kernel, np.float32)
    wkk = np.asarray(wk_kernel, np.float32)
    wvk = np.asarray(wv_kernel, np.float32)
    wok = np.asarray(wo_kernel, np.float32)

    def tile_x(a):  # [S, D] -> [P, TDIN, S] pre-tiled transpose
        return np.ascontiguousarray(
            a.T.reshape(TDIN, P, S).transpose(1, 0, 2)
        ).astype(BF16)

    xt = [[tile_x(x[b]) for x in (q, k, v)] for b in range(B)]
    mt = [
        np.ascontiguousarray(mask[b].T.astype(np.float32)).astype(BF16)
        for b in range(B)
    ]
    in_maps = []
    for c in range(NCORES):
        b, g = divmod(c, GH)
        cs = slice(g * DG, (g + 1) * DG)
        wo_arr = np.ascontiguousarray(
            wok[cs, :].reshape(HPG, HD, D).transpose(1, 0, 2)
        ).astype(BF16)
        in_maps.append(
            {
                "xq_t": xt[b][0],
                "xk_t": xt[b][1],
                "xv_t": xt[b][2],
                "mask_t": mt[b],
                "wq": np.ascontiguousarray(wqk[:, cs].reshape(TDIN, P, DG).transpose(1, 0, 2)).astype(BF16),
                "wk": np.ascontiguousarray(wkk[:, cs].reshape(TDIN, P, DG).transpose(1, 0, 2)).astype(BF16),
                "wv": np.ascontiguousarray(wvk[:, cs].reshape(TDIN, P, DG).transpose(1, 0, 2)).astype(BF16),
                "wo": wo_arr,
                "qb": np.asarray(wq_bias, np.float32)[cs].reshape(1, DG).astype(BF16),
                "kb": np.asarray(wk_bias, np.float32)[cs].reshape(1, DG).astype(BF16),
                "vb": np.asarray(wv_bias, np.float32)[cs].reshape(1, DG).astype(BF16),
            }
        )
    return in_maps


def combine_outputs(results, wo_bias):
    outs = np.stack([np.asarray(r["out"], np.float32) for r in results])
    full = outs.reshape(B, GH, S, D).sum(axis=1)
    return (full + np.asarray(wo_bias, np.float32)[None, None, :]).astype(
        np.float32
    )


def kernel(**inputs):
    from concourse import bass_utils

    nc = _graph()
    in_maps = make_in_maps(**inputs)
    res = bass_utils.run_bass_kernel_spmd(
        nc, in_maps, core_ids=list(range(NCORES))
    )
    return combine_outputs(res.results, inputs["wo_bias"])



# revision 5
# speedup vs baseline: 1.0644x; 1.0644x over previous
# Multi-head attention (B=2, S=2048, D=1024, H=16) on 8 TRN2 NeuronCores.
#
# Sharding (hardcoded): core c in [0..8) handles batch b = c//4 and head
# group g = c%4 (4 heads = 256 output features of wq/wk/wv, 256 input rows
# of wo). Each core computes a partial output projection [S, D]; the host
# sums the 4 partials per batch and adds the output bias (row-parallel).
#
# v2 schedule (single fused stream, every engine kept busy):
#   - attention processed in q-blocks of 512 over head PAIRS (2t, 2t+1);
#     the pair's score matmuls are K=64 row-tiles at partition 0 / 64 and
#     run concurrently in the PE array (tile_position auto-derivation);
#   - score PSUM slots [128, 2, 512] double-buffered so the scalar engine
#     streams exp() back-to-back (one [128,1024] ACTIVATE per k-tile);
#   - softmax denominator via a ones-column appended to each head's V;
#     normalization multiplies by a reciprocal broadcast bounced via DRAM;
#   - out-projection contracts K=128 over stacked head pairs (odd head
#     shifted to partitions 64..127 by a small SBUF->SBUF DMA) and is
#     emitted as filler between attention units, as are the projection
#     chunks for later q-blocks, keeping the PE dense and the HAM warm;
#   - q/k biases are folded into the PSUM->SBUF drains (tensor_scalar);
#     the v bias is folded into the host-side combine (vb @ wo).
import functools
import sys

import numpy as np

try:
    import concourse  # noqa: F401
except ImportError:  # harness env without the default path
    sys.path.insert(0, "/opt/trn_rl_repo")
    sys.path.insert(0, "/opt/pypackages")

import ml_dtypes

BF16 = ml_dtypes.bfloat16

B, S, D, H = 2, 2048, 1024, 16
HD = D // H          # 64
NCORES = 8
GH = 4               # head groups (tensor-parallel)
HPG = H // GH        # heads per group = 4
DG = D // GH         # features per group = 256
P = 128              # partitions
TDIN = D // P        # 8 din tiles
NT2 = DG // P        # 2 dout tiles per group (= head pairs)
KT = S // P          # 16 k tiles
QB = 512             # attention q block
NQB = S // QB        # 4
XC = 512             # projection s-chunk
NXC = S // XC        # 4


def build_graph():
    """Build the SPMD Bass graph (identical on all 8 cores)."""
    from contextlib import ExitStack

    from concourse import bacc, mybir, tile

    f32 = mybir.dt.float32
    bf16 = mybir.dt.bfloat16
    EXP = mybir.ActivationFunctionType.Exp

    nc = bacc.Bacc(
        "TRN2", target_bir_lowering=False, debug=False, num_devices=NCORES
    )

    xq = nc.dram_tensor("xq_t", (P, TDIN, S), bf16, kind="ExternalInput")
    xk = nc.dram_tensor("xk_t", (P, TDIN, S), bf16, kind="ExternalInput")
    xv = nc.dram_tensor("xv_t", (P, TDIN, S), bf16, kind="ExternalInput")
    mk = nc.dram_tensor("mask_t", (S, S), bf16, kind="ExternalInput")
    wq = nc.dram_tensor("wq", (P, TDIN, DG), bf16, kind="ExternalInput")
    wk = nc.dram_tensor("wk", (P, TDIN, DG), bf16, kind="ExternalInput")
    wv = nc.dram_tensor("wv", (P, TDIN, DG), bf16, kind="ExternalInput")
    # wo pre-arranged host-side to [128, 2, D]: partition p of pair t holds
    # row (2t + p//64)*64 + p%64 of the group's wo shard.
    wo = nc.dram_tensor("wo", (P, NT2, D), bf16, kind="ExternalInput")
    qb = nc.dram_tensor("qb", (P, NT2), f32, kind="ExternalInput")
    kb = nc.dram_tensor("kb", (P, NT2), f32, kind="ExternalInput")
    out = nc.dram_tensor("out", (S, D), bf16, kind="ExternalOutput")

    with tile.TileContext(nc) as tc, ExitStack() as ctx:
        wpool = ctx.enter_context(tc.tile_pool(name="wpool", bufs=1))
        cpool = ctx.enter_context(tc.tile_pool(name="cpool", bufs=1))
        qkpool = ctx.enter_context(tc.tile_pool(name="qk", bufs=1))
        vpool = ctx.enter_context(tc.tile_pool(name="vsb", bufs=1))
        mpool = ctx.enter_context(tc.tile_pool(name="msk", bufs=1))
        opool = ctx.enter_context(tc.tile_pool(name="otn", bufs=1))
        xpool = ctx.enter_context(tc.tile_pool(name="xin", bufs=3))
        ppool = ctx.enter_context(tc.tile_pool(name="ptile", bufs=3))
        npool = ctx.enter_context(tc.tile_pool(name="nrm", bufs=2))
        otpool = ctx.enter_context(tc.tile_pool(name="ottmp", bufs=2))
        obpool = ctx.enter_context(tc.tile_pool(name="outsb", bufs=2))
        dpool = ctx.enter_context(tc.tile_pool(name="dscr", bufs=2, space="DRAM"))
        slpool = ctx.enter_context(tc.tile_pool(name="slot", bufs=2, space="PSUM"))
        o_pool = ctx.enter_context(tc.tile_pool(name="oacc", bufs=3, space="PSUM"))
        sppool = ctx.enter_context(tc.tile_pool(name="sp", bufs=1, space="PSUM"))

        # ---- activation-table warmup (overlaps the DMA head) -------------
        warm = cpool.tile([1, 16], f32)
        nc.vector.memset(warm[:], 0.0)
        nc.scalar.activation(warm[:], warm[:], EXP, scale=1.0)

        # ---- persistent SBUF tensors -------------------------------------
        wq_sb = wpool.tile([P, TDIN, DG], bf16)
        wk_sb = wpool.tile([P, TDIN, DG], bf16)
        wv_sb = wpool.tile([P, TDIN, DG], bf16)
        wo_sb = wpool.tile([P, NT2, D], bf16)
        qb_sb = cpool.tile([P, NT2], f32)
        kb_sb = cpool.tile([P, NT2], f32)
        def load_w(wsb_, wdr_):
            for th_ in range(2):
                nc.sync.dma_start(
                    wsb_[:, th_ * 4 : (th_ + 1) * 4, :],
                    wdr_.ap()[:, th_ * 4 : (th_ + 1) * 4, :],
                )

        qT_sb = qkpool.tile([P, NT2, S], bf16)   # q projection, transposed
        kT_sb = qkpool.tile([P, NT2, S], bf16)
        # v blocks: per k-tile, per head: [v(64) | ones] -> 65 cols
        v_sb = vpool.tile([P, KT, HPG * (HD + 1)], bf16)
        nc.vector.memset(
            v_sb[:].rearrange("p s (h x) -> p s h x", h=HPG)[:, :, :, HD : HD + 1],
            1.0,
        )
        mask_sb = mpool.tile([P, KT, S], bf16)
        mk_r = mk.ap().rearrange("(t p) q -> p t q", p=P)
        otn_sb = opool.tile([P, NT2, S], bf16)

        # ---- input DMA emission (priority order = first-use order) -------
        xk_ch = [None] * NXC
        xv_ch = [None] * NXC
        xq_ch = [None] * NXC

        def load_x(xdram, c, tag):
            xch = xpool.tile([P, TDIN, XC], bf16, tag="xch", name=f"x_{tag}_{c}")
            for th_ in range(2):
                nc.sync.dma_start(
                    xch[:, th_ * 4 : (th_ + 1) * 4, :],
                    xdram.ap()[:, th_ * 4 : (th_ + 1) * 4, c * XC : (c + 1) * XC],
                )
            return xch

        def load_mask(qc):
            for kh in range(2):
                nc.sync.dma_start(
                    mask_sb[:, kh * 8 : (kh + 1) * 8, qc * QB : (qc + 1) * QB],
                    mk_r[:, kh * 8 : (kh + 1) * 8, qc * QB : (qc + 1) * QB],
                )

        # first-use order: k-proj c0 unblocks the first matmuls fastest
        load_w(wk_sb, wk)
        xk_ch[0] = load_x(xk, 0, "k")
        nc.sync.dma_start(kb_sb[:], kb.ap())
        load_w(wv_sb, wv)
        xv_ch[0] = load_x(xv, 0, "v")
        load_w(wq_sb, wq)
        xq_ch[0] = load_x(xq, 0, "q")
        nc.sync.dma_start(qb_sb[:], qb.ap())
        load_mask(0)
        nc.sync.dma_start(wo_sb[:], wo.ap())
        xk_ch[1] = load_x(xk, 1, "k")
        xv_ch[1] = load_x(xv, 1, "v")
        for c in range(2, NXC):
            xk_ch[c] = load_x(xk, c, "k")
            xv_ch[c] = load_x(xv, c, "v")

        # ---- projection chunk emitters -----------------------------------
        def emit_kq_chunk(xch, wsb, bias_sb, dest, c, tag):
            for dt in range(NT2):
                ps = sppool.tile([P, XC], f32, tag="sp", name=f"pj_{tag}_{c}_{dt}")
                for din in range(TDIN):
                    nc.tensor.matmul(
                        ps[:],
                        lhsT=wsb[:, din, dt * P : (dt + 1) * P],
                        rhs=xch[:, din, :],
                        start=(din == 0),
                        stop=(din == TDIN - 1),
                    )
                nc.vector.tensor_scalar_add(
                    dest[:, dt, c * XC : (c + 1) * XC], ps[:],
                    bias_sb[:, dt : dt + 1],
                )

        def emit_v_chunk(xch, c):
            for m in range(XC // P):
                st = c * (XC // P) + m
                ps = sppool.tile([P, XC], f32, tag="sp", name=f"pv_{c}_{m}")
                for din in range(TDIN):
                    nc.tensor.matmul(
                        ps[:, 0:DG],
                        lhsT=xch[:, din, m * P : (m + 1) * P],
                        rhs=wv_sb[:, din, :],
                        start=(din == 0),
                        stop=(din == TDIN - 1),
                    )
                nc.vector.tensor_copy(
                    v_sb[:, st, :].rearrange("p (h x) -> p h x", h=HPG)[
                        :, :, 0:HD
                    ],
                    ps[:, 0:DG].rearrange("p (h x) -> p h x", h=HPG),
                )

        # ---- out-projection emitter (one st = 128 output rows) -----------
        def emit_outproj(st):
            osb = obpool.tile([P, NT2, XC], bf16, tag="outsb", name=f"ou_{st}")
            for nch in range(2):
                ps = sppool.tile([P, XC], f32, tag="sp", name=f"po_{st}_{nch}")
                for t in range(NT2):
                    nc.tensor.matmul(
                        ps[:],
                        lhsT=otn_sb[:, t, st * P : (st + 1) * P],
                        rhs=wo_sb[:, t, nch * XC : (nch + 1) * XC],
                        start=(t == 0),
                        stop=(t == NT2 - 1),
                    )
                nc.vector.tensor_copy(osb[:, nch, :], ps[:])
            nc.sync.dma_start(out.ap()[st * P : (st + 1) * P, :], osb[:])

        # ---- prologue projections ----------------------------------------
        emit_kq_chunk(xk_ch[0], wk_sb, kb_sb, kT_sb, 0, "k")
        emit_v_chunk(xv_ch[0], 0)
        emit_kq_chunk(xq_ch[0], wq_sb, qb_sb, qT_sb, 0, "q")
        emit_kq_chunk(xk_ch[1], wk_sb, kb_sb, kT_sb, 1, "k")
        emit_v_chunk(xv_ch[1], 1)

        # ---- attention ---------------------------------------------------
        def emit_unit(qc, t, kt, o_ps):
            sl = slpool.tile([P, 2, QB], f32, tag="sl", name=f"sl_{qc}_{t}_{kt}")
            for half in range(2):
                po = half * HD
                nc.tensor.matmul(
                    sl[:, half, :],
                    lhsT=kT_sb[po : po + HD, t, kt * P : (kt + 1) * P],
                    rhs=qT_sb[po : po + HD, t, qc * QB : (qc + 1) * QB],
                    start=True,
                    stop=True,
                )
            pt = ppool.tile([P, 2, QB], bf16, tag="pt", name=f"pt_{qc}_{t}_{kt}")
            nc.scalar.activation(pt[:], sl[:], EXP, scale=0.125)
            for half in range(2):
                nc.vector.tensor_mul(
                    pt[:, half, :], pt[:, half, :],
                    mask_sb[:, kt, qc * QB : (qc + 1) * QB],
                )
            for half in range(2):
                h = 2 * t + half
                nc.tensor.matmul(
                    o_ps[half][:],
                    lhsT=v_sb[:, kt, h * 65 : (h + 1) * 65],
                    rhs=pt[:, half, :],
                    start=(kt == 0),
                    stop=(kt == KT - 1),
                )

        def emit_norm(qc, t, half, o_ps):
            # reciprocal must stay partition-aligned (DVE lanes cannot
            # shift partitions); DMA extracts the denominator row after.
            rec = npool.tile([HD + 1, QB], f32, tag="rec",
                             name=f"rc_{qc}_{t}_{half}")
            nc.vector.reciprocal_approx_fast(out=rec[:], in_=o_ps[:])
            scr = dpool.tile([1, QB], f32, tag="scr", name=f"sc_{qc}_{t}_{half}")
            nc.sync.dma_start(scr[:], rec[HD : HD + 1, :])
            rb = npool.tile([HD, QB], f32, tag="rb", name=f"rb_{qc}_{t}_{half}")
            nc.sync.dma_start(rb[:], scr[:].to_broadcast((HD, QB)))
            if half == 0:
                nc.vector.tensor_mul(
                    otn_sb[0:HD, t, qc * QB : (qc + 1) * QB],
                    o_ps[0:HD, :], rb[:],
                )
            else:
                ot = otpool.tile([HD, QB], bf16, tag="ot", name=f"ot_{qc}_{t}")
                nc.vector.tensor_mul(ot[:], o_ps[0:HD, :], rb[:])
                nc.sync.dma_start(
                    otn_sb[HD:P, t, qc * QB : (qc + 1) * QB], ot[:]
                )

        # filler work interleaved between attention units, per (qc, pair)
        def fillers_for(qc, t):
            fl = []
            if qc == 0 and t == 0:
                fl.append(lambda: emit_kq_chunk(xk_ch[2], wk_sb, kb_sb, kT_sb, 2, "k"))
                fl.append(lambda: emit_v_chunk(xv_ch[2], 2))
                fl.append(lambda: emit_kq_chunk(xk_ch[3], wk_sb, kb_sb, kT_sb, 3, "k"))
                fl.append(lambda: emit_v_chunk(xv_ch[3], 3))
            elif qc == 0 and t == 1:
                def ldq1():
                    xq_ch[1] = load_x(xq, 1, "q")
                fl.append(ldq1)
                fl.append(lambda: load_mask(1))
                fl.append(lambda: emit_kq_chunk(xq_ch[1], wq_sb, qb_sb, qT_sb, 1, "q"))
            else:
                # out-proj for q-block qc-1 (4 st tiles), split across pairs
                base = (qc - 1) * (QB // P)
                sts = [base, base + 1] if t == 0 else [base + 2, base + 3]
                for st in sts:
                    fl.append(lambda st=st: emit_outproj(st))
                if t == 0 and qc < NQB - 1:
                    def ldq(c=qc + 1):
                        xq_ch[c] = load_x(xq, c, "q")
                    fl.append(ldq)
                    fl.append(lambda c=qc + 1: load_mask(c))
                    fl.append(lambda c=qc + 1: emit_kq_chunk(
                        xq_ch[c], wq_sb, qb_sb, qT_sb, c, "q"))
            return fl

        for qc in range(NQB):
            for t in range(NT2):
                o_ps = [
                    o_pool.tile([HD + 1, QB], f32, tag="o",
                                name=f"o_{qc}_{t}_{half}")
                    for half in range(2)
                ]
                fl = fillers_for(qc, t)
                fi = 0
                for kt in range(KT):
                    emit_unit(qc, t, kt, o_ps)
                    # pump at most one filler every other unit
                    if kt % 2 == 1 and fi < len(fl):
                        fl[fi]()
                        fi += 1
                while fi < len(fl):
                    fl[fi]()
                    fi += 1
                for half in range(2):
                    emit_norm(qc, t, half, o_ps[half])

        # epilogue: out-proj for the last q-block
        for st in range((NQB - 1) * (QB // P), S // P):
            emit_outproj(st)

    nc.compile()
    return nc


@functools.lru_cache(maxsize=1)
def _graph():
    return build_graph()


def make_in_maps(
    query, key, value, mask,
    wq_kernel, wq_bias, wk_kernel, wk_bias,
    wv_kernel, wv_bias, wo_kernel, wo_bias,
):
    q = np.asarray(query, np.float32)
    k = np.asarray(key, np.float32)
    v = np.asarray(value, np.float32)
    mask = np.asarray(mask)
    wqk = np.asarray(wq_kernel, np.float32)
    wkk = np.asarray(wk_kernel, np.float32)
    wvk = np.asarray(wv_kernel, np.float32)
    wok = np.asarray(wo_kernel, np.float32)

    def tile_x(a):  # [S, D] -> [P, TDIN, S] pre-tiled transpose
        return np.ascontiguousarray(
            a.T.reshape(TDIN, P, S).transpose(1, 0, 2)
        ).astype(BF16)

    xt = [[tile_x(x[b]) for x in (q, k, v)] for b in range(B)]
    mt = [
        np.ascontiguousarray(mask[b].T.astype(np.float32)).astype(BF16)
        for b in range(B)
    ]
    in_maps = []
    for c in range(NCORES):
        b, g = divmod(c, GH)
        cs = slice(g * DG, (g + 1) * DG)
        w4 = wok[cs, :].reshape(HPG, HD, D)
        # [128, 2, D]: pair t stacks heads (2t, 2t+1) along partitions
        wo_arr = np.ascontiguousarray(np.stack(
            [np.concatenate([w4[2 * t], w4[2 * t + 1]], axis=0)
             for t in range(NT2)], axis=1,
        )).astype(BF16)
        in_maps.append(
            {
                "xq_t": xt[b][0],
                "xk_t": xt[b][1],
                "xv_t": xt[b][2],
                "mask_t": mt[b],
                "wq": np.ascontiguousarray(wqk[:, cs].reshape(TDIN, P, DG).transpose(1, 0, 2)).astype(BF16),
                "wk": np.ascontiguousarray(wkk[:, cs].reshape(TDIN, P, DG).transpose(1, 0, 2)).astype(BF16),
                "wv": np.ascontiguousarray(wvk[:, cs].reshape(TDIN, P, DG).transpose(1, 0, 2)).astype(BF16),
                "wo": wo_arr,
                "qb": np.ascontiguousarray(
                    np.asarray(wq_bias, np.float32)[cs].reshape(NT2, P).T),
                "kb": np.ascontiguousarray(
                    np.asarray(wk_bias, np.float32)[cs].reshape(NT2, P).T),
            }
        )
    return in_maps


def combine_outputs(results, wo_bias, wv_bias, wo_kernel):
    outs = np.stack([np.asarray(r["out"], np.float32) for r in results])
    full = outs.reshape(B, GH, S, D).sum(axis=1)
    # v-bias folded here: attn(v + vb) = attn(v) + vb, then @ wo
    bias = np.asarray(wo_bias, np.float32) + (
        np.asarray(wv_bias, np.float32) @ np.asarray(wo_kernel, np.float32)
    )
    return (full + bias[None, None, :]).astype(np.float32)


def kernel(**inputs):
    from concourse import bass_utils

    nc = _graph()
    in_maps = make_in_maps(**inputs)
    res = bass_utils.run_bass_kernel_spmd(
        nc, in_maps, core_ids=list(range(NCORES))
    )
    return combine_outputs(
        res.results, inputs["wo_bias"], inputs["wv_bias"], inputs["wo_kernel"]
    )
